# revision 8
# baseline (speedup 1.0000x reference)
"""Trainium2 Bass kernel for nn_Network_77464030151182 (gnn_message_passing).

Strategy (self-contained; shapes hardcoded):
  - 512 populations sharded 64/core across 8 NeuronCores; no collectives.
  - Per core, SBUF partition q = h*64 + p covers grid half h (4096 cols) of
    local pop p.  The TVD stencil runs chunked along the grid axis with a
    2-left/1-right halo.
  - Synapses are packed by postsynaptic population into a [128, WCOL] layout
    (each pop's synapse list split across its two partitions), so the
    segment sums become free-axis reductions; a tiny constant matmul
    (pair matrix M[k,m] = 1 iff k%64==m%64) folds the two partial sums per
    pop and broadcasts the result to both grid-half partitions.
  - SRpre = ro[pre_idx, 0] is gathered host-side during input packing.
"""
import sys

sys.path.insert(0, "/opt/trn_rl_repo")

import numpy as np
import concourse.bass as bass
import concourse.bacc as bacc
import concourse.mybir as mybir
from concourse import tile
from concourse import bass_utils

P, N, S = 512, 8192, 262144
NC = 8
PPC = P // NC            # 64 pops per core
HALF = N // 2            # 4096
F = 1024                 # stencil chunk columns per partition
NCHUNK = HALF // F

DT, DTS = 0.1, 0.5
VT, EL, CMEM, GL = -50.0, -60.0, 1.0, 0.1
SQRT2 = float(np.sqrt(2.0, dtype=np.float32))
SQRT_2_PI = 0.7978845608028654
SIGMA_EFF = 0.3 / 0.1 * float(np.sqrt(0.5 * 0.1 / 1.0))
K_T = float(np.float32(1.0 / (SIGMA_EFF * SQRT2)))
C_LIM = 0.5 * (1.0 - DT / DTS)                   # 0.4
A4 = -0.0117
S1 = float(np.float32(-0.072 / -0.0117))
S2 = float(np.float32(-0.257 / -0.0117))
S3 = float(np.float32(-1.12 / -0.0117))
Q0 = float(np.float32(0.0061 / -0.0117))

f32 = mybir.dt.float32
AF = mybir.ActivationFunctionType
OP = mybir.AluOpType

SYN_NAMES = ["Xp", "Yp", "Up", "tdp", "trp", "tfp", "uip", "gbp", "erp", "wp", "srp"]


def build_module(wcol):
    nc = bacc.Bacc("TRN2", target_bir_lowering=False, debug=False)

    syn_in = {
        n: nc.dram_tensor(n, [128, wcol], f32, kind="ExternalInput")
        for n in SYN_NAMES
    }
    V_d = nc.dram_tensor("V", [PPC, N], f32, kind="ExternalInput")
    ro_d = nc.dram_tensor("ro", [PPC, N], f32, kind="ExternalInput")
    iext_d = nc.dram_tensor("iext", [128, 1], f32, kind="ExternalInput")
    pairM_d = nc.dram_tensor("pairM", [128, 128], f32, kind="ExternalInput")
    dX_d = nc.dram_tensor("dX", [128, wcol], f32, kind="ExternalOutput")
    dY_d = nc.dram_tensor("dY", [128, wcol], f32, kind="ExternalOutput")
    dU_d = nc.dram_tensor("dU", [128, wcol], f32, kind="ExternalOutput")
    dro_d = nc.dram_tensor("dro", [PPC, N], f32, kind="ExternalOutput")
    dV_d = nc.dram_tensor("dV", [PPC, N], f32, kind="ExternalOutput")

    with tile.TileContext(nc) as tc:
        with (
            tc.tile_pool(name="const", bufs=1) as cpool,
            tc.tile_pool(name="syn", bufs=1) as spool,
            tc.tile_pool(name="io", bufs=2) as iopool,
            tc.tile_pool(name="work", bufs=1) as wpool,
            tc.tile_pool(name="chain", bufs=2) as hpool,
            tc.tile_pool(name="psum", bufs=1, space="PSUM") as ppool,
        ):
            # ---------------- constants ----------------
            pairM_t = cpool.tile([128, 128], f32, name="pairM", tag="pairM")
            nc.sync.dma_start(pairM_t[:], pairM_d[:])
            iext_t = cpool.tile([128, 1], f32, name="iext", tag="iext")
            nc.sync.dma_start(iext_t[:], iext_d[:])

            # ---------------- synapse phase ----------------
            st = {}
            for n in SYN_NAMES:
                st[n] = spool.tile([128, wcol], f32, name=n, tag=n)
                nc.sync.dma_start(st[n][:], syn_in[n][:])

            def stile(tag):
                return spool.tile([128, wcol], f32, name=tag, tag=tag)

            d_t = stile("d")
            nc.vector.tensor_sub(d_t[:], st["tdp"][:], st["trp"][:])
            rd_t = stile("rd")
            nc.vector.reciprocal(rd_t[:], d_t[:])
            tau1r = stile("tau1r")
            nc.vector.tensor_mul(tau1r[:], st["tdp"][:], rd_t[:])
            mask_t = spool.tile([128, wcol], mybir.dt.uint8, name="mask", tag="mask")
            nc.vector.tensor_scalar(mask_t[:], d_t[:], 0.0, None, OP.is_equal)
            c13_t = stile("c13")
            nc.vector.memset(c13_t[:], 1e-13)
            nc.vector.copy_predicated(tau1r[:], mask_t[:], c13_t[:])

            # e_d/e_r/e_f = exp(-DT/tau); reuse rd_t/d_t/mask_t slots
            e_t = {}
            for tau, tag in (("tdp", "ed"), ("trp", "er_"), ("tfp", "ef")):
                rc = stile(tag + "r")
                nc.vector.reciprocal(rc[:], st[tau][:])
                e_t[tag] = stile(tag)
                nc.scalar.activation(e_t[tag][:], rc[:], AF.Exp, scale=-DT)
            ed, er_, ef = e_t["ed"], e_t["er_"], e_t["ef"]

            y_ = stile("y_")
            nc.vector.tensor_mul(y_[:], st["Yp"][:], ed[:])
            ty = stile("ty")
            nc.vector.tensor_mul(ty[:], tau1r[:], st["Yp"][:])
            q1 = stile("q1")
            nc.vector.scalar_tensor_tensor(q1[:], st["Xp"][:], -1.0, ty[:], OP.add, OP.add)
            q2 = stile("q2")
            nc.vector.tensor_mul(q2[:], q1[:], er_[:])
            q3 = stile("q3")
            nc.vector.tensor_sub(q3[:], q2[:], ty[:])
            x_ = stile("x_")
            nc.scalar.activation(x_[:], q3[:], AF.Identity, bias=1.0)
            u_ = stile("u_")
            nc.vector.tensor_mul(u_[:], st["Up"][:], ef[:])
            t_ = stile("t_")
            nc.vector.tensor_scalar(t_[:], u_[:], -1.0, 1.0, OP.mult, OP.add)
            t2_ = stile("t2_")
            nc.vector.tensor_mul(t2_[:], t_[:], st["uip"][:])
            t3_ = stile("t3_")
            nc.vector.tensor_mul(t3_[:], t2_[:], st["srp"][:])
            u0 = stile("u0")
            nc.vector.tensor_add(u0[:], u_[:], t3_[:])
            ux = stile("ux")
            nc.vector.tensor_mul(ux[:], u0[:], x_[:])
            qq = stile("qq")
            nc.vector.tensor_mul(qq[:], ux[:], st["srp"][:])

            # dX = ((x_ - qq) - X)*10 etc.
            x0 = stile("x0")
            nc.vector.tensor_sub(x0[:], x_[:], qq[:])
            dd1 = stile("dd1")
            nc.vector.tensor_sub(dd1[:], x0[:], st["Xp"][:])
            dXt = stile("dXt")
            nc.scalar.activation(dXt[:], dd1[:], AF.Identity, scale=1.0 / DT)
            nc.sync.dma_start(dX_d[:], dXt[:])

            y0 = stile("y0")
            nc.vector.tensor_add(y0[:], y_[:], qq[:])
            dd2 = stile("dd2")
            nc.vector.tensor_sub(dd2[:], y0[:], st["Yp"][:])
            dYt = stile("dYt")
            nc.scalar.activation(dYt[:], dd2[:], AF.Identity, scale=1.0 / DT)
            nc.sync.dma_start(dY_d[:], dYt[:])

            dd3 = stile("dd3")
            nc.vector.tensor_sub(dd3[:], u0[:], st["Up"][:])
            dUt = stile("dUt")
            nc.scalar.activation(dUt[:], dd3[:], AF.Identity, scale=1.0 / DT)
            nc.sync.dma_start(dU_d[:], dUt[:])

            # segment sums (per-partition partials via accum_out)
            wg = stile("wg")
            nc.vector.tensor_mul(wg[:], st["wp"][:], st["gbp"][:])
            rhs2 = cpool.tile([128, 2], f32, name="rhs2", tag="rhs2")
            gsyn = stile("gsyn")
            nc.vector.scalar_tensor_tensor(
                gsyn[:], wg[:], 0.0, st["Yp"][:], OP.add, OP.mult,
                accum_out=rhs2[:, 0:1])
            gEt = stile("gEt")
            nc.vector.scalar_tensor_tensor(
                gEt[:], gsyn[:], 0.0, st["erp"][:], OP.add, OP.mult,
                accum_out=rhs2[:, 1:2])

            psum2 = ppool.tile([128, 2], f32, name="psum2", tag="psum2")
            nc.tensor.matmul(psum2[:], lhsT=pairM_t[:], rhs=rhs2[:],
                             start=True, stop=True)

            b_t = cpool.tile([128, 1], f32, name="b", tag="b")
            nc.vector.tensor_scalar_add(b_t[:], psum2[:, 0:1], GL)
            a_t = cpool.tile([128, 1], f32, name="a", tag="a")
            nc.vector.scalar_tensor_tensor(
                a_t[:], psum2[:, 1:2], GL * EL, iext_t[:], OP.add, OP.add)
            rb_t = cpool.tile([128, 1], f32, name="rb", tag="rb")
            nc.vector.reciprocal(rb_t[:], b_t[:])
            taumB = cpool.tile([128, 1], f32, name="taumB", tag="taumB")
            nc.vector.tensor_scalar_mul(taumB[:], rb_t[:], -SQRT2 * SQRT_2_PI)
            negb = cpool.tile([128, 1], f32, name="negb", tag="negb")
            nc.vector.tensor_scalar_mul(negb[:], b_t[:], -1.0)

            f_acc = cpool.tile([128, 1], f32, name="f_acc", tag="f_acc")
            nc.vector.memset(f_acc[:], 0.0)
            ro0_t = cpool.tile([128, 1], f32, name="ro0", tag="ro0")
            biasT = cpool.tile([128, 1], f32, name="biasT", tag="biasT")
            nc.vector.memset(biasT[:], VT * K_T)
            biasA = cpool.tile([128, 1], f32, name="biasA", tag="biasA")
            nc.vector.memset(biasA[:], A4 * Q0)

            # ---------------- population phase ----------------
            for kk in range(NCHUNK):
                base = kk * F
                first, last = kk == 0, kk == NCHUNK - 1

                zV = iopool.tile([128, F + 3], f32, name="zV", tag="zV")
                zR = iopool.tile([128, F + 3], f32, name="zR", tag="zR")
                for z_t, src_d in ((zV, V_d), (zR, ro_d)):
                    if first:
                        nc.sync.dma_start(z_t[0:64, 2:F + 3], src_d[:, 0:F + 1])
                        nc.scalar.copy(z_t[0:64, 0:1], z_t[0:64, 2:3])
                        nc.scalar.copy(z_t[0:64, 1:2], z_t[0:64, 2:3])
                    else:
                        nc.sync.dma_start(
                            z_t[0:64, :], src_d[:, base - 2:base + F + 1])
                    if last:
                        nc.sync.dma_start(
                            z_t[64:128, 0:F + 2],
                            src_d[:, HALF + base - 2:N])
                        nc.scalar.copy(z_t[64:128, F + 2:F + 3],
                                       z_t[64:128, F + 1:F + 2])
                    else:
                        nc.sync.dma_start(
                            z_t[64:128, :],
                            src_d[:, HALF + base - 2:HALF + base + F + 1])

                if first:
                    nc.scalar.copy(ro0_t[0:64, :], zR[0:64, 2:3])

                Vc = zV[:, 2:F + 2]
                Rc = zR[:, 2:F + 2]

                dvdt = hpool.tile([128, F], f32, name="dvdt", tag="dvdt")
                nc.scalar.activation(dvdt[:], Vc, AF.Identity,
                                     scale=negb[:], bias=a_t[:])
                Tt = hpool.tile([128, F], f32, name="Tt", tag="Tt")
                nc.scalar.activation(Tt[:], Vc, AF.Identity,
                                     scale=-K_T, bias=biasT[:])
                nc.vector.tensor_scalar_max(Tt[:], Tt[:], -K_T)
                wa = hpool.tile([128, F], f32, name="wa", tag="wa")
                nc.vector.scalar_tensor_tensor(wa[:], Tt[:], S1, Tt[:], OP.add, OP.mult)
                wb = hpool.tile([128, F], f32, name="wb", tag="wb")
                nc.vector.scalar_tensor_tensor(wb[:], wa[:], S2, Tt[:], OP.add, OP.mult)
                nc.vector.scalar_tensor_tensor(wa[:], wb[:], S3, Tt[:], OP.add, OP.mult)
                A_t = hpool.tile([128, F], f32, name="A", tag="A")
                nc.scalar.activation(A_t[:], wa[:], AF.Exp, scale=A4, bias=biasA[:])
                T2 = hpool.tile([128, F], f32, name="T2", tag="T2")
                nc.scalar.activation(T2[:], Tt[:], AF.Square)
                nc.scalar.activation(T2[:], T2[:], AF.Exp, scale=-1.0)
                erf = hpool.tile([128, F], f32, name="erf", tag="erf")
                nc.scalar.activation(erf[:], Tt[:], AF.Erf)
                nc.vector.tensor_scalar_add(erf[:], erf[:], 1.00000001)
                nc.vector.reciprocal(erf[:], erf[:])
                nc.vector.tensor_mul(T2[:], T2[:], erf[:])       # T2 = F_T'
                nc.vector.tensor_scalar(wb[:], dvdt[:], -K_T, 0.0, OP.mult, OP.min)
                nc.vector.tensor_mul(wa[:], wb[:], T2[:])        # wa = Bp
                nc.vector.tensor_scalar_mul(wa[:], wa[:], taumB[:])
                nc.vector.tensor_add(A_t[:], A_t[:], wa[:])
                nc.vector.tensor_scalar(A_t[:], A_t[:], b_t[:], 0.0, OP.mult, OP.max)
                SRC = hpool.tile([128, F], f32, name="SRC", tag="SRC")
                acc_c = wpool.tile([128, 1], f32, name="acc_c", tag="acc_c")
                nc.vector.scalar_tensor_tensor(
                    SRC[:], Rc, 0.0, A_t[:], OP.add, OP.mult, accum_out=acc_c[:])
                nc.vector.tensor_add(f_acc[:], f_acc[:], acc_c[:])

                def stencil(z_t, src_ap, sub_src, out_d, zkind):
                    D = wpool.tile([128, F + 2], f32, name="D" + zkind, tag="D" + zkind)
                    nc.vector.tensor_sub(D[:], z_t[:, 1:F + 3], z_t[:, 0:F + 2])
                    S_ = wpool.tile([128, F + 1], f32, name="S" + zkind, tag="S" + zkind)
                    nc.vector.tensor_add(S_[:], D[:, 1:F + 2], D[:, 0:F + 1])
                    nc.scalar.activation(S_[:], S_[:], AF.Abs, scale=0.5)
                    AD = wpool.tile([128, F + 2], f32, name="AD" + zkind, tag="AD" + zkind)
                    nc.scalar.activation(AD[:], D[:], AF.Abs, scale=2.0)
                    X2 = wpool.tile([128, F + 1], f32, name="X2" + zkind, tag="X2" + zkind)
                    nc.vector.tensor_tensor(
                        X2[:], AD[:, 1:F + 2], AD[:, 0:F + 1], OP.min)
                    nc.vector.tensor_tensor(X2[:], S_[:], X2[:], OP.min)  # WI
                    nc.vector.tensor_sub(
                        S_[:, 0:F], X2[:, 1:F + 1], X2[:, 0:F])           # WD
                    nc.vector.scalar_tensor_tensor(
                        AD[:, 0:F], S_[:, 0:F], C_LIM / DTS, src_ap,
                        OP.mult, OP.subtract if sub_src else OP.add)      # t1
                    DZ = iopool.tile([128, F], f32, name="DZ" + zkind, tag="DZ" + zkind)
                    nc.vector.scalar_tensor_tensor(
                        DZ[:], D[:, 1:F + 1], -1.0 / DTS, AD[:, 0:F],
                        OP.mult, OP.subtract)
                    return DZ, X2

                DZr, WIr = stencil(zR, SRC[:], False, dro_d, "r")
                DZv, WIv = stencil(zV, dvdt[:], True, dV_d, "v")

                if first:
                    nc.vector.memset(DZv[0:64, 0:1], 0.0)
                if last:
                    fixt = wpool.tile([128, 1], f32, name="fixt", tag="fixt")
                    nc.vector.scalar_tensor_tensor(
                        fixt[64:128, :], WIr[64:128, F - 1:F], C_LIM,
                        zR[64:128, F:F + 1], OP.mult, OP.add)
                    nc.vector.scalar_tensor_tensor(
                        DZr[64:128, F - 1:F], fixt[64:128, :], 1.0 / DTS,
                        SRC[64:128, F - 1:F], OP.mult, OP.subtract)
                    nc.scalar.copy(DZv[64:128, F - 1:F], dvdt[64:128, F - 1:F])

                for DZ, out_d in ((DZr, dro_d), (DZv, dV_d)):
                    if first and DZ is DZr:
                        nc.sync.dma_start(out_d[:, 1:F], DZ[0:64, 1:F])
                    else:
                        nc.sync.dma_start(out_d[:, base:base + F], DZ[0:64, :])
                    nc.sync.dma_start(
                        out_d[:, HALF + base:HALF + base + F], DZ[64:128, :])

            # firing fixup: dro[:, 0] = -ro0/DTS + firing
            psumf = ppool.tile([128, 1], f32, name="psumf", tag="psumf")
            nc.tensor.matmul(psumf[:], lhsT=pairM_t[:], rhs=f_acc[:],
                             start=True, stop=True)
            dro0 = cpool.tile([128, 1], f32, name="dro0", tag="dro0")
            nc.vector.scalar_tensor_tensor(
                dro0[0:64, :], ro0_t[0:64, :], -1.0 / DTS, psumf[0:64, :],
                OP.mult, OP.add)
            nc.sync.dma_start(dro_d[:, 0:1], dro0[0:64, :])

    nc.compile()
    return nc


_CACHE = {}


def _get_module(wcol):
    if wcol not in _CACHE:
        _CACHE[wcol] = build_module(wcol)
    return _CACHE[wcol]


def _pack_meta(post_idx, wpad):
    order = np.argsort(post_idx, kind="stable")
    posts = post_idx[order]
    counts = np.bincount(post_idx, minlength=P)
    starts = np.zeros(P + 1, np.int64)
    np.cumsum(counts, out=starts[1:])
    rank = np.arange(S, dtype=np.int64) - starts[posts]
    pos = np.full((P, wpad), -1, np.int64)
    pos[posts, rank] = order
    return pos


def _to_layout(a):
    """[PPC, WPAD] -> [128, WCOL], partition q = h*64 + p."""
    ppc, wpad = a.shape
    wcol = wpad // 2
    return np.ascontiguousarray(
        a.reshape(ppc, 2, wcol).transpose(1, 0, 2).reshape(2 * ppc, wcol))


def host_prep(inputs):
    X = inputs["X"]; Ysyn = inputs["Ysyn"]; U = inputs["U"]
    ro = inputs["ro"]; V = inputs["V"]
    tau_d = inputs["tau_d"]; tau_r = inputs["tau_r"]; tau_f = inputs["tau_f"]
    Uinc = inputs["Uinc"]; gbarS = inputs["gbarS"]; Erev = inputs["Erev"]
    W = inputs["W"]; Iext = inputs["Iext"]
    pre_idx = inputs["pre_idx"]; post_idx = inputs["post_idx"]

    counts_max = int(np.bincount(post_idx, minlength=P).max())
    wpad = max(640, (counts_max + 127) // 128 * 128)
    wcol = wpad // 2
    pos = _pack_meta(post_idx, wpad)

    SRpre = ro[pre_idx, 0].astype(np.float32)

    kidx = np.arange(128)
    pairM = (kidx[:, None] % 64 == kidx[None, :] % 64).astype(np.float32)

    fills = {"Xp": 0.0, "Yp": 0.0, "Up": 0.0, "tdp": 2.0, "trp": 1.0,
             "tfp": 1.0, "uip": 0.0, "gbp": 0.0, "erp": 0.0, "wp": 0.0,
             "srp": 0.0}
    full = {"Xp": X, "Yp": Ysyn, "Up": U, "tdp": tau_d, "trp": tau_r,
            "tfp": tau_f, "uip": Uinc, "gbp": gbarS, "erp": Erev, "wp": W,
            "srp": SRpre}

    in_maps = []
    pos_lays = []
    for c in range(NC):
        psl = slice(c * PPC, (c + 1) * PPC)
        pos_c = pos[psl]
        m_c = pos_c >= 0
        im = {}
        for name in SYN_NAMES:
            buf = np.full((PPC, wpad), fills[name], np.float32)
            buf[m_c] = full[name][pos_c[m_c]]
            im[name] = _to_layout(buf)
        im["V"] = np.ascontiguousarray(V[psl], dtype=np.float32)
        im["ro"] = np.ascontiguousarray(ro[psl], dtype=np.float32)
        im["iext"] = np.ascontiguousarray(
            np.tile(Iext[psl].astype(np.float32), 2)[:, None])
        im["pairM"] = pairM
        in_maps.append(im)
        pos_lays.append(_to_layout(pos_c))

    return in_maps, pos_lays, wcol


def assemble(results, pos_lays):
    dX = np.empty(S, np.float32)
    dY = np.empty(S, np.float32)
    dU = np.empty(S, np.float32)
    dro = np.empty((P, N), np.float32)
    dV = np.empty((P, N), np.float32)
    for c in range(NC):
        psl = slice(c * PPC, (c + 1) * PPC)
        r = results[c]
        lay = pos_lays[c]
        m = lay >= 0
        dX[lay[m]] = r["dX"][m]
        dY[lay[m]] = r["dY"][m]
        dU[lay[m]] = r["dU"][m]
        dro[psl] = r["dro"]
        dV[psl] = r["dV"]

    return np.concatenate([dX, dY, dU, dro.reshape(-1), dV.reshape(-1)])


def kernel(**inputs):
    in_maps, pos_lays, wcol = host_prep(inputs)
    nc = _get_module(wcol)
    res = bass_utils.run_bass_kernel_spmd(nc, in_maps, list(range(NC)))
    return assemble(res.results, pos_lays)
